# revision 5
# baseline (speedup 1.0000x reference)
"""CrossAttention kernel for Trainium2 (8 NeuronCores, SPMD).

Reference math (B=4, C=256, N=4096, OUT=256, TEMP=sqrt(OUT)=16):
    q = Wq @ x; k = Wk @ xx; v = Wv @ xx        (B, OUT, N)
    attn = softmax(q^T k / TEMP, axis=-1)       (B, N, N)
    y = einsum('bnm,bom->bon', attn, v)         (B, OUT, N)

Sharding: 8 cores = (batch b, query-half h); each core computes its 2048
query rows against the full 4096 keys of its batch.

Per-core algorithm (v2: G-folding + centered-fp8 PV):
    host: g = Wk^T Wq / TEMP  (so S = x^T (g^T xx) needs no q-projection)
          sv = Wv @ sum_m(xx) (exact softmax-correction column sums)
    t   = g^T @ xx            (C, m)   bf16 PE, fp32 psum
    v8  = fp8e4(Wv @ xx)      [m-part, o] layout
    S_T = t-tiles^T @ x       (m, n) blocks, bf16, fp32 psum
    P   = exp(S_T)            ACT -> bf16 staging
    P8  = fp8e4(P - 1)        DVE centering (kills fp8 noise on the O(1)
                              mean; exact correction via +sv below)
    y[o,n] = (sum_m v8[m,o] P8[m,n] + sv[o]) / (sum_m P8[m,n] + 4096)
        PV: v8-stationary DoubleRow fp8 matmuls (2 m-tiles per MM)
        den: ones-stationary DoubleRow matmuls -> [1, n]
        1/den broadcast across partitions, DVE normalize
    y comes out directly in (OUT, n) layout: no PE transposes.
"""

import numpy as np
import ml_dtypes
from contextlib import ExitStack

import concourse.bass as bass
import concourse.tile as tile
from concourse import bacc, mybir
from concourse.bass_utils import run_bass_kernel_spmd

B, C, NSEQ, OUT = 4, 256, 4096, 256
TEMP = float(OUT) ** 0.5
NCORES = 8
BF16 = mybir.dt.bfloat16
F32 = mybir.dt.float32
FP8 = mybir.dt.float8e4
DR = mybir.MatmulPerfMode.DoubleRow
BFNP = ml_dtypes.bfloat16
E4NP = ml_dtypes.float8_e4m3

EXP = mybir.ActivationFunctionType.Exp


def build(bc=2048, m=4096, nblk=512, repeat_full=1):
    """Build the per-core SPMD Bass program.

    bc: query rows per core; m: key count; nblk: query block width
    (nblk*4B <= one PSUM bank). repeat_full: re-run the whole body R times
    (perf measurement only).
    """
    ct = C // 128     # contraction tiles
    mt = m // 128     # key tiles
    nb = bc // nblk   # query blocks
    kch = m // 512    # xkv 512-chunks

    nc = bacc.Bacc("TRN2", target_bir_lowering=False, debug=False,
                   num_devices=NCORES)
    xq_d = nc.dram_tensor("xq", [ct, 128, bc], BF16, kind="ExternalInput")
    xkv_d = nc.dram_tensor("xkv", [ct, 128, m], BF16, kind="ExternalInput")
    g_d = nc.dram_tensor("g", [ct, 128, C], BF16, kind="ExternalInput")
    wv_d = nc.dram_tensor("wvT", [ct, 128, OUT], BF16, kind="ExternalInput")
    sv_d = nc.dram_tensor("sv", [OUT // 128, 128, 1], F32,
                          kind="ExternalInput")
    y_d = nc.dram_tensor("y", [OUT // 128, 128, bc], F32,
                         kind="ExternalOutput")

    with tile.TileContext(nc) as tc, ExitStack() as ctx:
        const = ctx.enter_context(tc.tile_pool(name="const", bufs=1))

        xq_sb = const.tile([128, ct, bc], BF16, name="xq_sb")
        xkv_sb = const.tile([128, ct, m], BF16, name="xkv_sb")
        g_sb = const.tile([128, ct, C], BF16, name="g_sb")
        wv_sb = const.tile([128, ct, OUT], BF16, name="wv_sb")
        sv_sb = const.tile([128, 2, 1], F32, name="sv_sb")
        t_sb = const.tile([128, ct, m], BF16, name="t_sb")
        v8_sb = const.tile([128, mt, OUT], FP8, name="v8_sb")
        y_sb = const.tile([128, 2, bc], F32, name="y_sb")
        ones8 = const.tile([128, 2, 16], FP8, name="ones8")
        ones1 = const.tile([1, 128], F32, name="ones1")
        zbias = const.tile([128, 1], F32, name="zbias")

        for _rf in range(repeat_full):
            # weights first (first matmuls need them); xkv chunked so t/v
            # projections start as soon as each chunk lands; xq afterwards
            for i in range(ct):
                nc.sync.dma_start(g_sb[:, i, :], g_d.ap()[i])
                nc.sync.dma_start(wv_sb[:, i, :], wv_d.ap()[i])
            for chk in range(kch):
                for i in range(ct):
                    nc.sync.dma_start(
                        xkv_sb[:, i, chk * 512:(chk + 1) * 512],
                        xkv_d.ap()[i][:, chk * 512:(chk + 1) * 512])
            for chk in range(bc // 512):
                for i in range(ct):
                    nc.sync.dma_start(
                        xq_sb[:, i, chk * 512:(chk + 1) * 512],
                        xq_d.ap()[i][:, chk * 512:(chk + 1) * 512])
            for h in range(2):
                nc.sync.dma_start(sv_sb[:, h, :], sv_d.ap()[h])
            nc.vector.memset(ones8[:], 1.0)
            nc.vector.memset(ones1[:], 1.0)
            nc.vector.memset(zbias[:], 0.0)

            # ---- t = g^T xx and v8 = fp8(Wv xx) projections ----
            with tc.tile_pool(name="t_ps", bufs=3, space="PSUM") as t_pool, \
                 tc.tile_pool(name="v_ps", bufs=3, space="PSUM") as v_pool:
                for chk in range(kch):
                    csl = slice(chk * 512, (chk + 1) * 512)
                    for ci in range(ct):
                        ps = t_pool.tile([128, 512], F32, tag="t", name="t_t")
                        for dh in range(ct):
                            nc.tensor.matmul(
                                ps[:], g_sb[:, dh, ci * 128:(ci + 1) * 128],
                                xkv_sb[:, dh, csl],
                                start=(dh == 0), stop=(dh == ct - 1))
                        nc.vector.tensor_copy(t_sb[:, ci, csl], ps[:])
                    for mi in range(4 * chk, 4 * (chk + 1)):
                        ps = v_pool.tile([128, OUT], F32, tag="v", name="v_t")
                        for dh in range(ct):
                            nc.tensor.matmul(
                                ps[:], xkv_sb[:, dh, mi * 128:(mi + 1) * 128],
                                wv_sb[:, dh, :],
                                start=(dh == 0), stop=(dh == ct - 1))
                        nc.vector.tensor_copy(v8_sb[:, mi, :], ps[:])

            # ---- attention ----
            with tc.tile_pool(name="s_ps", bufs=3, space="PSUM") as s_pool, \
                 tc.tile_pool(name="y_ps", bufs=2, space="PSUM") as y_pool, \
                 tc.tile_pool(name="d_ps", bufs=1, space="PSUM") as d_pool, \
                 tc.tile_pool(name="p8", bufs=3) as p8_pool, \
                 tc.tile_pool(name="pst", bufs=4) as pst_pool, \
                 tc.tile_pool(name="fin", bufs=3) as fin_pool:
                P8_tiles = [None] * nb

                def s_stage(blk):
                    n0 = blk * nblk
                    P8 = p8_pool.tile([128, mt, nblk], FP8, tag="p8",
                                      name="P8")
                    P8_tiles[blk] = P8
                    for mi in range(mt):
                        s_ps = s_pool.tile([128, nblk], F32, tag="s",
                                           name="s_t")
                        for ch in range(ct):
                            nc.tensor.matmul(
                                s_ps[:], t_sb[:, ch, mi * 128:(mi + 1) * 128],
                                xq_sb[:, ch, n0:n0 + nblk],
                                start=(ch == 0), stop=(ch == ct - 1))
                        pst = pst_pool.tile([128, nblk], BF16, tag="pst",
                                            name="pst")
                        nc.scalar.activation(pst[:], s_ps[:], EXP,
                                             bias=zbias[:], scale=1.0)
                        nc.vector.tensor_scalar_add(P8[:, mi, :], pst[:],
                                                    -1.0)

                def pv_stage(blk):
                    n0 = blk * nblk
                    P8 = P8_tiles[blk]
                    den_ps = d_pool.tile([1, nblk], F32, tag="d", name="d_t")
                    yps = [y_pool.tile([128, nblk], F32, tag=f"y{oh}",
                                       name="y_t") for oh in range(2)]
                    for j in range(mt // 2):
                        rhs = P8[:, 2 * j:2 * j + 2, :]
                        nc.tensor.matmul(den_ps[:], ones8[:, :, j:j + 1],
                                         rhs, start=(j == 0),
                                         stop=(j == mt // 2 - 1),
                                         perf_mode=DR)
                        for oh in range(2):
                            nc.tensor.matmul(
                                yps[oh][:],
                                v8_sb[:, 2 * j:2 * j + 2,
                                      oh * 128:(oh + 1) * 128],
                                rhs, start=(j == 0),
                                stop=(j == mt // 2 - 1), perf_mode=DR)
                    den = fin_pool.tile([1, nblk], F32, tag="den",
                                        name="den")
                    nc.vector.tensor_scalar_add(den[:], den_ps[:], float(m))
                    recip = fin_pool.tile([1, nblk], F32, tag="recip",
                                          name="recip")
                    nc.vector.reciprocal(recip[:], den[:])
                    dinv_ps = s_pool.tile([128, nblk], F32, tag="s",
                                          name="dinv")
                    nc.tensor.matmul(dinv_ps[:], ones1[:], recip[:],
                                     start=True, stop=True)
                    for oh in range(2):
                        ysl = y_sb[:, oh, n0:n0 + nblk]
                        yt = fin_pool.tile([128, nblk], F32, tag="yt",
                                           name="yt")
                        nc.vector.tensor_scalar_add(yt[:], yps[oh][:],
                                                    sv_sb[:, oh, :])
                        nc.vector.tensor_tensor(ysl, yt[:], dinv_ps[:],
                                                mybir.AluOpType.mult)
                        nc.sync.dma_start(y_d.ap()[oh][:, n0:n0 + nblk], ysl)

                s_stage(0)
                s_stage(1)
                pv_stage(0)
                s_stage(2)
                pv_stage(1)
                s_stage(3)
                pv_stage(2)
                pv_stage(3)
    nc.compile()
    return nc


def make_in_maps(x, xx, Wq, Wk, Wv, bc=2048, m=4096):
    """Host-side prep: slice/cast per-core inputs. Returns list of 8 dicts."""
    ct = C // 128
    g = (Wk.T.astype(np.float64) @ Wq.astype(np.float64) / TEMP
         ).astype(np.float32)
    g_t = np.ascontiguousarray(g.astype(BFNP).reshape(ct, 128, C))
    wv_t = np.ascontiguousarray(Wv.T.astype(BFNP).reshape(ct, 128, OUT))
    halves = NCORES // B
    in_maps = []
    for core in range(NCORES):
        b, h = divmod(core, halves)
        xq = np.ascontiguousarray(
            x[b, :, h * bc:(h + 1) * bc].astype(BFNP).reshape(ct, 128, bc))
        xkv = np.ascontiguousarray(
            xx[b, :, :m].astype(BFNP).reshape(ct, 128, m))
        sv = (Wv.astype(np.float64) @ xx[b].astype(np.float64).sum(-1)
              ).astype(np.float32).reshape(OUT // 128, 128, 1)
        in_maps.append({"xq": xq, "xkv": xkv, "g": g_t, "wvT": wv_t,
                        "sv": sv})
    return in_maps


def gather_output(results, bc=2048):
    """Reassemble per-core (2, 128, bc) outputs into (B, OUT, NSEQ)."""
    y = np.empty((B, OUT, NSEQ), dtype=np.float32)
    halves = NCORES // B
    for core, res in enumerate(results):
        b, h = divmod(core, halves)
        y[b, :, h * bc:(h + 1) * bc] = res["y"].reshape(OUT, bc)
    return y


_NC_CACHE = {}


def kernel(x, xx, Wq, Wk, Wv):
    x = np.asarray(x)
    xx = np.asarray(xx)
    key = "full"
    if key not in _NC_CACHE:
        _NC_CACHE[key] = build()
    nc = _NC_CACHE[key]
    in_maps = make_in_maps(x, xx, np.asarray(Wq), np.asarray(Wk),
                           np.asarray(Wv))
    try:
        res = run_bass_kernel_spmd(nc, in_maps, core_ids=list(range(NCORES)))
    except Exception:
        # transient device state (e.g. a previous process left a core
        # unrecoverable) usually clears on retry
        res = run_bass_kernel_spmd(nc, in_maps, core_ids=list(range(NCORES)))
    return gather_output(res.results)


# revision 7
# speedup vs baseline: 1.0045x; 1.0045x over previous
"""CrossAttention kernel for Trainium2 (8 NeuronCores, SPMD).

Reference math (B=4, C=256, N=4096, OUT=256, TEMP=sqrt(OUT)=16):
    q = Wq @ x; k = Wk @ xx; v = Wv @ xx        (B, OUT, N)
    attn = softmax(q^T k / TEMP, axis=-1)       (B, N, N)
    y = einsum('bnm,bom->bon', attn, v)         (B, OUT, N)

Sharding: 8 cores = (batch b, query-half h); each core computes its 2048
query rows against the full 4096 keys of its batch.

Per-core algorithm (all matmuls bf16 in / fp32 PSUM accumulate):
    host: g = Wk^T Wq / TEMP   (folds the q-projection away:
                                S = x^T (g^T xx), no q needed)
    t   = g^T @ xx             (C, m)      [c on partitions]
    vT  = xx^T @ Wv^T (+ones col) -> (m, OUT+1)  [m on partitions]
    S_T = t-tiles^T @ x        (m, n) blocks     [m on partitions]
    P   = exp(S_T) bf16 (logits are O(3) so no max-subtraction needed)
    yT  = P-tiles^T @ vT_aug -> (128n, OUT+1) psum; last col = denom
    yn  = yT[:, :OUT] / yT[:, OUT]  -> DMA'd as (n, OUT) tiles;
          the final (OUT, n) transpose happens on the host.

The S+exp stage of block b+1 is emitted before the PV stage of block b
so ScalarE's exp hides under PE matmuls.
"""

import numpy as np
import ml_dtypes
from contextlib import ExitStack

import concourse.bass as bass
import concourse.tile as tile
from concourse import bacc, mybir
from concourse.bass_utils import run_bass_kernel_spmd

B, C, NSEQ, OUT = 4, 256, 4096, 256
TEMP = float(OUT) ** 0.5
NCORES = 8
BF16 = mybir.dt.bfloat16
F32 = mybir.dt.float32
BFNP = ml_dtypes.bfloat16

EXP = mybir.ActivationFunctionType.Exp


def build(bc=2048, m=4096, nblk=512, repeat_full=1):
    """Build the per-core SPMD Bass program.

    bc: query rows per core; m: key count; nblk: query block width
    (nblk*4B <= one PSUM bank). repeat_full: re-run the whole body R times
    (perf measurement only).
    """
    ct = C // 128     # contraction tiles
    mt = m // 128     # key tiles
    nb = bc // nblk   # query blocks
    nt = nblk // 128  # 128-query tiles per block
    kch = m // 512    # xkv 512-chunks

    nc = bacc.Bacc("TRN2", target_bir_lowering=False, debug=False,
                   num_devices=NCORES)
    xq_d = nc.dram_tensor("xq", [ct, 128, bc], BF16, kind="ExternalInput")
    xkv_d = nc.dram_tensor("xkv", [ct, 128, m], BF16, kind="ExternalInput")
    g_d = nc.dram_tensor("g", [ct, 128, C], BF16, kind="ExternalInput")
    wv_d = nc.dram_tensor("wvT", [ct, 128, OUT], BF16, kind="ExternalInput")
    y_d = nc.dram_tensor("y", [bc // 128, 128, OUT], F32,
                         kind="ExternalOutput")

    with tile.TileContext(nc) as tc, ExitStack() as ctx:
        const = ctx.enter_context(tc.tile_pool(name="const", bufs=1))

        xq_sb = const.tile([128, ct, bc], BF16, name="xq_sb")
        xkv_sb = const.tile([128, ct, m], BF16, name="xkv_sb")
        g_sb = const.tile([128, ct, C], BF16, name="g_sb")
        wv_sb = const.tile([128, ct, OUT], BF16, name="wv_sb")
        t_sb = const.tile([128, ct, m], BF16, name="t_sb")
        v_sb = const.tile([128, mt, OUT + 1], BF16, name="v_sb")
        zbias = const.tile([128, 1], F32, name="zbias")

        for _rf in range(repeat_full):
            # weights first (first matmuls need them); xkv chunked so t/v
            # projections start as soon as each chunk lands; xq afterwards
            for i in range(ct):
                nc.sync.dma_start(g_sb[:, i, :], g_d.ap()[i])
                nc.sync.dma_start(wv_sb[:, i, :], wv_d.ap()[i])
            for chk in range(kch):
                for i in range(ct):
                    nc.sync.dma_start(
                        xkv_sb[:, i, chk * 512:(chk + 1) * 512],
                        xkv_d.ap()[i][:, chk * 512:(chk + 1) * 512])
            for chk in range(bc // 512):
                for i in range(ct):
                    nc.sync.dma_start(
                        xq_sb[:, i, chk * 512:(chk + 1) * 512],
                        xq_d.ap()[i][:, chk * 512:(chk + 1) * 512])
            nc.vector.memset(zbias[:], 0.0)
            nc.vector.memset(v_sb[:, :, OUT:OUT + 1], 1.0)

            # ---- t = g^T xx and vT (+ones col) projections ----
            with tc.tile_pool(name="t_ps", bufs=3, space="PSUM") as t_pool, \
                 tc.tile_pool(name="v_ps", bufs=3, space="PSUM") as v_pool:
                for chk in range(kch):
                    csl = slice(chk * 512, (chk + 1) * 512)
                    for ci in range(ct):
                        ps = t_pool.tile([128, 512], F32, tag="t", name="t_t")
                        for dh in range(ct):
                            nc.tensor.matmul(
                                ps[:], g_sb[:, dh, ci * 128:(ci + 1) * 128],
                                xkv_sb[:, dh, csl],
                                start=(dh == 0), stop=(dh == ct - 1))
                        nc.vector.tensor_copy(t_sb[:, ci, csl], ps[:])
                    for mi in range(4 * chk, 4 * (chk + 1)):
                        ps = v_pool.tile([128, OUT], F32, tag="v", name="v_t")
                        for dh in range(ct):
                            nc.tensor.matmul(
                                ps[:], xkv_sb[:, dh, mi * 128:(mi + 1) * 128],
                                wv_sb[:, dh, :],
                                start=(dh == 0), stop=(dh == ct - 1))
                        # ACT (idle here; all Copy-activates land before the
                        # first Exp, so only one table switch)
                        nc.scalar.copy(v_sb[:, mi, 0:OUT], ps[:])

            # ---- attention ----
            with tc.tile_pool(name="p_sb", bufs=2) as p_pool, \
                 tc.tile_pool(name="s_ps", bufs=2, space="PSUM") as s_pool, \
                 tc.tile_pool(name="y_ps", bufs=4, space="PSUM") as y_pool, \
                 tc.tile_pool(name="fin", bufs=3) as fin_pool:
                P_tiles = [None] * nb

                def s_stage(blk):
                    # S_T = t^T x for block blk, exp -> P
                    # m-tiles paired: one [128, 2, nblk] psum tile (2 banks),
                    # one exp per pair
                    n0 = blk * nblk
                    P_sb = p_pool.tile([128, mt, nblk], BF16, tag="p",
                                       name="P_sb")
                    P_tiles[blk] = P_sb
                    for mj in range(mt // 2):
                        s_ps = s_pool.tile([128, 2, nblk], F32, tag="s",
                                           name="s_t")
                        for half in range(2):
                            mi = 2 * mj + half
                            for ch in range(ct):
                                nc.tensor.matmul(
                                    s_ps[:, half, :],
                                    t_sb[:, ch, mi * 128:(mi + 1) * 128],
                                    xq_sb[:, ch, n0:n0 + nblk],
                                    start=(ch == 0), stop=(ch == ct - 1))
                        nc.scalar.activation(
                            P_sb[:, 2 * mj:2 * mj + 2, :], s_ps[:], EXP,
                            bias=zbias[:], scale=1.0)

                def pv_stage(blk):
                    # yT = P^T v_aug; normalize by the ones-column; DMA out
                    # in (n, OUT) layout -- host does the final transpose
                    P_sb = P_tiles[blk]
                    for ni in range(nt):
                        y_ps = y_pool.tile([128, OUT + 1], F32, tag="y",
                                           name="y_t")
                        for mi in range(mt):
                            nc.tensor.matmul(
                                y_ps[:],
                                P_sb[:, mi, ni * 128:(ni + 1) * 128],
                                v_sb[:, mi, :],
                                start=(mi == 0), stop=(mi == mt - 1))
                        recip = fin_pool.tile([128, 1], F32, tag="recip",
                                              name="recip")
                        nc.vector.reciprocal(recip[:], y_ps[:, OUT:OUT + 1])
                        yn = fin_pool.tile([128, OUT], F32, tag="yn",
                                           name="yn", bufs=8)
                        nc.vector.tensor_scalar_mul(yn[:], y_ps[:, 0:OUT],
                                                    recip[:])
                        gni = blk * nt + ni
                        nc.sync.dma_start(y_d.ap()[gni], yn[:])

                s_stage(0)
                s_stage(1)
                pv_stage(0)
                s_stage(2)
                pv_stage(1)
                s_stage(3)
                pv_stage(2)
                pv_stage(3)
    nc.compile()
    return nc


def make_in_maps(x, xx, Wq, Wk, Wv, bc=2048, m=4096):
    """Host-side prep: slice/cast per-core inputs. Returns list of 8 dicts."""
    ct = C // 128
    g = (Wk.T.astype(np.float64) @ Wq.astype(np.float64) / TEMP
         ).astype(np.float32)
    g_t = np.ascontiguousarray(g.astype(BFNP).reshape(ct, 128, C))
    wv_t = np.ascontiguousarray(Wv.T.astype(BFNP).reshape(ct, 128, OUT))
    halves = NCORES // B
    in_maps = []
    for core in range(NCORES):
        b, h = divmod(core, halves)
        xq = np.ascontiguousarray(
            x[b, :, h * bc:(h + 1) * bc].astype(BFNP).reshape(ct, 128, bc))
        xkv = np.ascontiguousarray(
            xx[b, :, :m].astype(BFNP).reshape(ct, 128, m))
        in_maps.append({"xq": xq, "xkv": xkv, "g": g_t, "wvT": wv_t})
    return in_maps


def gather_output(results, bc=2048):
    """Reassemble per-core (bc/128, 128, OUT) outputs into (B, OUT, NSEQ)."""
    y = np.empty((B, OUT, NSEQ), dtype=np.float32)
    halves = NCORES // B
    for core, res in enumerate(results):
        b, h = divmod(core, halves)
        yc = res["y"]  # (bc/128, 128, OUT): [n-tile, n-in-tile, o]
        y[b, :, h * bc:(h + 1) * bc] = yc.reshape(bc, OUT).T
    return y


_NC_CACHE = {}


def kernel(x, xx, Wq, Wk, Wv):
    x = np.asarray(x)
    xx = np.asarray(xx)
    key = "full"
    if key not in _NC_CACHE:
        _NC_CACHE[key] = build()
    nc = _NC_CACHE[key]
    in_maps = make_in_maps(x, xx, np.asarray(Wq), np.asarray(Wk),
                           np.asarray(Wv))
    try:
        res = run_bass_kernel_spmd(nc, in_maps, core_ids=list(range(NCORES)))
    except Exception:
        # transient device state (e.g. a previous process left a core
        # unrecoverable) usually clears on retry
        res = run_bass_kernel_spmd(nc, in_maps, core_ids=list(range(NCORES)))
    return gather_output(res.results)


# revision 11
# speedup vs baseline: 1.1575x; 1.1523x over previous
"""CrossAttention kernel for Trainium2 (8 NeuronCores, SPMD).

Reference math (B=4, C=256, N=4096, OUT=256, TEMP=sqrt(OUT)=16):
    q = Wq @ x; k = Wk @ xx; v = Wv @ xx        (B, OUT, N)
    attn = softmax(q^T k / TEMP, axis=-1)       (B, N, N)
    y = einsum('bnm,bom->bon', attn, v)         (B, OUT, N)

Sharding: 8 cores = (batch b, query-half h); each core computes its 2048
query rows against the full 4096 keys of its batch.

Per-core algorithm (all matmuls bf16 in / fp32 PSUM accumulate):
    host: g = Wk^T Wq / TEMP   (folds the q-projection away:
                                S = x^T (g^T xx), no q needed)
    t   = g^T @ xx             (C, m)      [c on partitions]
    vT  = xx^T @ Wv^T (+ones col) -> (m, OUT+1)  [m on partitions]
    S_T = t-tiles^T @ x        (m, n) blocks     [m on partitions]
    P   = exp(S_T) bf16 (logits are O(3) so no max-subtraction needed)
    yT  = P-tiles^T @ vT_aug -> (128n, OUT+1) psum; last col = denom
    yn  = yT[:, :OUT] / yT[:, OUT]  -> DMA'd as (n, OUT) tiles;
          the final (OUT, n) transpose happens on the host.

The S+exp stage of block b+1 is emitted before the PV stage of block b
so ScalarE's exp hides under PE matmuls.
"""

import numpy as np
import ml_dtypes
from contextlib import ExitStack

import concourse.bass as bass
import concourse.tile as tile
from concourse import bacc, mybir
from concourse.bass_utils import run_bass_kernel_spmd

B, C, NSEQ, OUT = 4, 256, 4096, 256
TEMP = float(OUT) ** 0.5
NCORES = 8
BF16 = mybir.dt.bfloat16
F32 = mybir.dt.float32
BFNP = ml_dtypes.bfloat16

EXP = mybir.ActivationFunctionType.Exp


def build(bc=2048, m=4096, nblk=512, repeat_full=1, sbufs=2, ybufs=4):
    """Build the per-core SPMD Bass program.

    bc: query rows per core; m: key count; nblk: query block width
    (nblk*4B <= one PSUM bank). repeat_full: re-run the whole body R times
    (perf measurement only).
    """
    ct = C // 128     # contraction tiles
    mt = m // 128     # key tiles
    nb = bc // nblk   # query blocks
    nt = nblk // 128  # 128-query tiles per block
    kch = m // 512    # xkv 512-chunks

    nc = bacc.Bacc("TRN2", target_bir_lowering=False, debug=False,
                   num_devices=NCORES)
    xq_d = nc.dram_tensor("xq", [ct, 128, bc], BF16, kind="ExternalInput")
    xkv_d = nc.dram_tensor("xkv", [ct, 128, m], BF16, kind="ExternalInput")
    g_d = nc.dram_tensor("g", [ct, 128, C], BF16, kind="ExternalInput")
    wv_d = nc.dram_tensor("wvT", [ct, 128, OUT], BF16, kind="ExternalInput")
    y_d = nc.dram_tensor("y", [bc // 128, 128, OUT], F32,
                         kind="ExternalOutput")

    with tile.TileContext(nc) as tc, ExitStack() as ctx:
        const = ctx.enter_context(tc.tile_pool(name="const", bufs=1))

        xq_sb = const.tile([128, ct, bc], BF16, name="xq_sb")
        xkv_sb = const.tile([128, ct, m], BF16, name="xkv_sb")
        g_sb = const.tile([128, ct, C], BF16, name="g_sb")
        wv_sb = const.tile([128, ct, OUT], BF16, name="wv_sb")
        t_sb = const.tile([128, ct, m], BF16, name="t_sb")
        v_sb = const.tile([128, mt, OUT + 1], BF16, name="v_sb")
        zbias = const.tile([128, 1], F32, name="zbias")

        for _rf in range(repeat_full):
            # weights first (first matmuls need them); xkv chunked so t/v
            # projections start as soon as each chunk lands; xq afterwards
            for i in range(ct):
                nc.sync.dma_start(g_sb[:, i, :], g_d.ap()[i])
                nc.sync.dma_start(wv_sb[:, i, :], wv_d.ap()[i])
            for chk in range(kch):
                for i in range(ct):
                    nc.sync.dma_start(
                        xkv_sb[:, i, chk * 512:(chk + 1) * 512],
                        xkv_d.ap()[i][:, chk * 512:(chk + 1) * 512])
            for chk in range(bc // 512):
                for i in range(ct):
                    nc.sync.dma_start(
                        xq_sb[:, i, chk * 512:(chk + 1) * 512],
                        xq_d.ap()[i][:, chk * 512:(chk + 1) * 512])
            nc.vector.memset(zbias[:], 0.0)
            nc.vector.memset(v_sb[:, :, OUT:OUT + 1], 1.0)

            # ---- t = g^T xx and vT (+ones col) projections ----
            with tc.tile_pool(name="t_ps", bufs=3, space="PSUM") as t_pool, \
                 tc.tile_pool(name="v_ps", bufs=3, space="PSUM") as v_pool:
                for chk in range(kch):
                    csl = slice(chk * 512, (chk + 1) * 512)
                    for ci in range(ct):
                        ps = t_pool.tile([128, 512], F32, tag="t", name="t_t")
                        for dh in range(ct):
                            nc.tensor.matmul(
                                ps[:], g_sb[:, dh, ci * 128:(ci + 1) * 128],
                                xkv_sb[:, dh, csl],
                                start=(dh == 0), stop=(dh == ct - 1))
                        nc.vector.tensor_copy(t_sb[:, ci, csl], ps[:])
                    for mi in range(4 * chk, 4 * (chk + 1)):
                        ps = v_pool.tile([128, OUT], F32, tag="v", name="v_t")
                        for dh in range(ct):
                            nc.tensor.matmul(
                                ps[:], xkv_sb[:, dh, mi * 128:(mi + 1) * 128],
                                wv_sb[:, dh, :],
                                start=(dh == 0), stop=(dh == ct - 1))
                        # ACT (idle here; all Copy-activates land before the
                        # first Exp, so only one table switch)
                        nc.scalar.copy(v_sb[:, mi, 0:OUT], ps[:])

            # ---- attention ----
            with tc.tile_pool(name="p_sb", bufs=2) as p_pool, \
                 tc.tile_pool(name="s_ps", bufs=sbufs, space="PSUM") as s_pool, \
                 tc.tile_pool(name="y_ps", bufs=ybufs, space="PSUM") as y_pool, \
                 tc.tile_pool(name="fin", bufs=3) as fin_pool:
                P_tiles = [None] * nb

                def s_stage(blk):
                    # S_T = t^T x for block blk, exp -> P
                    # m-tiles paired: one [128, 2, nblk] psum tile (2 banks),
                    # one exp per pair
                    n0 = blk * nblk
                    P_sb = p_pool.tile([128, mt, nblk], BF16, tag="p",
                                       name="P_sb")
                    P_tiles[blk] = P_sb
                    for mj in range(mt // 2):
                        s_ps = s_pool.tile([128, 2, nblk], F32, tag="s",
                                           name="s_t")
                        for half in range(2):
                            mi = 2 * mj + half
                            for ch in range(ct):
                                nc.tensor.matmul(
                                    s_ps[:, half, :],
                                    t_sb[:, ch, mi * 128:(mi + 1) * 128],
                                    xq_sb[:, ch, n0:n0 + nblk],
                                    start=(ch == 0), stop=(ch == ct - 1))
                        nc.scalar.activation(
                            P_sb[:, 2 * mj:2 * mj + 2, :], s_ps[:], EXP,
                            bias=zbias[:], scale=1.0)

                def pv_stage(blk):
                    # yT = P^T v_aug; normalize by the ones-column; DMA out
                    # in (n, OUT) layout -- host does the final transpose
                    P_sb = P_tiles[blk]
                    for ni in range(nt):
                        y_ps = y_pool.tile([128, OUT + 1], F32, tag="y",
                                           name="y_t")
                        for mi in range(mt):
                            nc.tensor.matmul(
                                y_ps[:],
                                P_sb[:, mi, ni * 128:(ni + 1) * 128],
                                v_sb[:, mi, :],
                                start=(mi == 0), stop=(mi == mt - 1))
                        recip = fin_pool.tile([128, 1], F32, tag="recip",
                                              name="recip")
                        nc.vector.reciprocal(recip[:], y_ps[:, OUT:OUT + 1])
                        yn = fin_pool.tile([128, OUT], F32, tag="yn",
                                           name="yn", bufs=8)
                        nc.vector.tensor_scalar_mul(yn[:], y_ps[:, 0:OUT],
                                                    recip[:])
                        gni = blk * nt + ni
                        nc.sync.dma_start(y_d.ap()[gni], yn[:])

                s_stage(0)
                s_stage(1)
                pv_stage(0)
                s_stage(2)
                pv_stage(1)
                s_stage(3)
                pv_stage(2)
                pv_stage(3)
    nc.compile()
    return nc


def make_in_maps(x, xx, Wq, Wk, Wv, bc=2048, m=4096):
    """Host-side prep: slice/cast per-core inputs. Returns list of 8 dicts."""
    ct = C // 128
    g = (Wk.T.astype(np.float64) @ Wq.astype(np.float64) / TEMP
         ).astype(np.float32)
    g_t = np.ascontiguousarray(g.astype(BFNP).reshape(ct, 128, C))
    wv_t = np.ascontiguousarray(Wv.T.astype(BFNP).reshape(ct, 128, OUT))
    halves = NCORES // B
    in_maps = []
    for core in range(NCORES):
        b, h = divmod(core, halves)
        xq = np.ascontiguousarray(
            x[b, :, h * bc:(h + 1) * bc].astype(BFNP).reshape(ct, 128, bc))
        xkv = np.ascontiguousarray(
            xx[b, :, :m].astype(BFNP).reshape(ct, 128, m))
        in_maps.append({"xq": xq, "xkv": xkv, "g": g_t, "wvT": wv_t})
    return in_maps


def gather_output(results, bc=2048):
    """Reassemble per-core (bc/128, 128, OUT) outputs into (B, OUT, NSEQ)."""
    y = np.empty((B, OUT, NSEQ), dtype=np.float32)
    halves = NCORES // B
    for core, res in enumerate(results):
        b, h = divmod(core, halves)
        yc = res["y"]  # (bc/128, 128, OUT): [n-tile, n-in-tile, o]
        y[b, :, h * bc:(h + 1) * bc] = yc.reshape(bc, OUT).T
    return y


_NC_CACHE = {}


def kernel(x, xx, Wq, Wk, Wv):
    x = np.asarray(x)
    xx = np.asarray(xx)
    key = "full"
    if key not in _NC_CACHE:
        _NC_CACHE[key] = build()
    nc = _NC_CACHE[key]
    in_maps = make_in_maps(x, xx, np.asarray(Wq), np.asarray(Wk),
                           np.asarray(Wv))
    try:
        res = run_bass_kernel_spmd(nc, in_maps, core_ids=list(range(NCORES)))
    except Exception:
        # transient device state (e.g. a previous process left a core
        # unrecoverable) usually clears on retry
        res = run_bass_kernel_spmd(nc, in_maps, core_ids=list(range(NCORES)))
    return gather_output(res.results)
